# revision 1
# baseline (speedup 1.0000x reference)
"""Trainium2 Bass kernel for nn_BayerUpsample4x4.

The reference op: x [4,1,1024,1024] -> 16-channel polyphase 4x bilinear
(tent-filter) upsample, output [4,16,1024,1024].  Each output channel
k=(r,c) is x subsampled at rows≡r, cols≡c (mod 4), zero-upsampled x4 and
convolved with the separable 7x7 tent kernel == bilinear interpolation
with zero padding at image borders.

Kernel plan (per core; 8 cores = 4 batches x 2 row-halves):
  - vertical interpolation on TensorE: fp32 matmul with banded interp
    matrices V (built host-side from `weight`), K=68 subsampled rows
  - PSUM evacuation fused with prescaling on ScalarE: P25/P50/P75
    = 0.25/0.5/0.75 * (vertical result), with 4 zero-pad cols both sides
  - horizontal interpolation as plain adds (measured-optimal split
    between VectorE and GpSimd):  e1 = P75_lo + P25_hi,
    e2 = P50_lo + P50_hi,  e3 = P25_lo + P75_hi
  - e0 columns = 2 * P50 (exact in fp32) on ScalarE
  - final stores are dense 512KB DMAs
"""

import sys
for _p in ("/opt/trn_rl_repo", "/opt/pypackages"):
    if _p not in sys.path:
        sys.path.append(_p)

from contextlib import ExitStack

import numpy as np

import concourse.bass as bass
import concourse.tile as tile
from concourse import bacc, mybir
from concourse.bass_utils import run_bass_kernel_spmd

F32 = mybir.dt.float32
AF = mybir.ActivationFunctionType
OP = mybir.AluOpType

N_CORES = 8
H, W = 1024, 1024
HALF = 512               # output rows per core
SLAB = 528               # padded input slab rows per core
KDIM = 68                # matmul contraction size (subsampled rows + halo)

# (row, col) offset within each 4x4 block for channel k (matches reference)
OFFSETS = [(0, 0), (0, 2), (2, 0), (2, 2),
           (0, 1), (0, 3), (2, 1), (2, 3),
           (1, 0), (1, 2), (3, 0), (3, 2),
           (1, 1), (1, 3), (3, 1), (3, 3)]
K_OF = {rc: k for k, rc in enumerate(OFFSETS)}

# calibrated per-op ns on HW, in-context (FD=256 strided fp32)
_COST_DVE_TT = 550.0
_COST_GPS_TT = 2000.0
_COST_ACT_E0 = 620.0
_COST_ACT_PRE = 550.0


def _emit(tc, xs, vm, out, kh, *, store=True, use_gps=False,
          bufs=(4, 3, 10), qs=(0, 1)):
    """Trace the per-core program.

    xs:  [528, 1024] f32 zero-padded input slab (rows h0-4 .. h0+523)
    vm:  [8, 68, 128] f32 vertical interp matrices, index r*2+b, [p, m]
    out: [16, 512, 1024] f32
    kh:  length-7 horizontal filter profile (numpy)
    """
    nc = tc.nc
    b_e = {e: float(kh[7 - e]) for e in (1, 2, 3)}   # 0.25 / 0.5 / 0.75

    load = {"dve": 0.0, "gps": 0.0, "act": 0.0}   # greedy engine balance
    if not use_gps:
        load["gps"] = 1e12
    setno = 0   # tile-set counter (for one-time pad init per pool slot)

    with ExitStack() as ctx:
        vpool = ctx.enter_context(tc.tile_pool(name="vmp", bufs=1))
        xpool = ctx.enter_context(tc.tile_pool(name="xp", bufs=5))
        pspool = ctx.enter_context(tc.tile_pool(name="psp", bufs=bufs[0],
                                                space="PSUM"))
        vtpool = ctx.enter_context(tc.tile_pool(name="vtp", bufs=bufs[1]))
        opool = ctx.enter_context(tc.tile_pool(name="op", bufs=bufs[2]))

        # ---- load all 8 V matrices into one [68, 8*128] tile ----
        vmt = vpool.tile([KDIM, 8 * 128], F32, tag="vmt")
        nc.sync.dma_start(vmt[:], vm.rearrange("i p m -> p i m"))

        xs_rows = xs.rearrange("(t s) w -> s t w", s=4)   # [4, 132, 1024]

        for q in qs:
            for r in range(4):
                xt = xpool.tile([KDIM, W], F32, tag="xt")
                nc.sync.dma_start(xt[:], xs_rows[r][64 * q: 64 * q + KDIM, :])

                for b in range(2):
                    lhsT = vmt[:, (r * 2 + b) * 128: (r * 2 + b + 1) * 128]

                    # prescaled vertical results; 4 zero pad cols both sides
                    p25 = vtpool.tile([128, W + 8], F32, tag="p25")
                    p50 = vtpool.tile([128, W + 8], F32, tag="p50")
                    p75 = vtpool.tile([128, W + 8], F32, tag="p75")
                    for t in (p25, p50, p75):
                        pad = t.rearrange("p (g u) -> p g u", u=4)
                        nc.vector.memset(pad[:, 0:258:257, :], 0.0)
                    setno += 1

                    pss = []
                    for ch in range(2):
                        ps = pspool.tile([128, 512], F32, tag="ps")
                        nc.tensor.matmul(
                            ps[:], lhsT=lhsT,
                            rhs=xt[:, 512 * ch: 512 * ch + 512],
                            start=True, stop=True,
                        )
                        pss.append(ps)
                    # p50 first across both chunks: e0/e2 consumers depend
                    # only on it and can start after two ACT ops
                    for scale, arr in ((b_e[2], p50), (b_e[1], p25),
                                       (b_e[3], p75)):
                        for ch in range(2):
                            dl = slice(4 + 512 * ch, 4 + 512 * ch + 512)
                            nc.scalar.activation(arr[:, dl], pss[ch][:],
                                                 AF.Copy, scale=scale)
                            load["act"] += _COST_ACT_PRE

                    # grouped [128, 258, 4] views for phase-strided access
                    pv = {1: p25.rearrange("p (u s) -> p u s", s=4),
                          2: p50.rearrange("p (u s) -> p u s", s=4),
                          3: p75.rearrange("p (u s) -> p u s", s=4)}

                    for c in range(4):
                        k = K_OF[(r, c)]
                        oc = opool.tile([128, W], F32, tag="oc")
                        ov = oc.rearrange("p (u s) -> p u s", s=4)
                        # e = 0: out phase c = Vt = 2*P50 (P50+P50 as TT keeps
                        # DVE/GpSimd in 1-port mode -> no shared-port lock)
                        u0, s0 = divmod(4 + c, 4)
                        src = pv[2][:, u0:u0 + 256, s0]
                        picks = {"act": load["act"] + _COST_ACT_E0,
                                 "dve": load["dve"] + _COST_DVE_TT,
                                 "gps": load["gps"] + _COST_GPS_TT}
                        eng = min(picks, key=picks.get)
                        load[eng] = picks[eng]
                        if eng == "act":
                            nc.scalar.activation(ov[:, :, c], src,
                                                 AF.Copy, scale=2.0)
                        elif eng == "dve":
                            nc.vector.tensor_tensor(ov[:, :, c], src, src,
                                                    OP.add)
                        else:
                            nc.gpsimd.tensor_tensor(ov[:, :, c], src, src,
                                                    OP.add)
                        for e in (1, 2, 3):
                            j0 = (c + e) % 4
                            st = 4 + j0 - e          # lo col start (1..6)
                            u0, s0 = divmod(st, 4)
                            u1, s1 = divmod(st + 4, 4)
                            lo = pv[4 - e][:, u0:u0 + 256, s0]
                            hi = pv[e][:, u1:u1 + 256, s1]
                            if load["dve"] + _COST_DVE_TT <= \
                                    load["gps"] + _COST_GPS_TT:
                                load["dve"] += _COST_DVE_TT
                                eng2 = nc.vector
                            else:
                                load["gps"] += _COST_GPS_TT
                                eng2 = nc.gpsimd
                            eng2.tensor_tensor(ov[:, :, j0], lo, hi, OP.add)
                        if store:
                            row0 = 256 * q + 128 * b
                            nc.sync.dma_start(out[k, row0:row0 + 128, :],
                                              oc[:])


_CACHE = {}


def _build_module(kh):
    key = tuple(np.asarray(kh, np.float64).tolist())
    if key in _CACHE:
        return _CACHE[key]
    nc = bacc.Bacc("TRN2", target_bir_lowering=False, debug=False)
    xs = nc.dram_tensor("xs", [SLAB, W], F32, kind="ExternalInput").ap()
    vm = nc.dram_tensor("vm", [8, KDIM, 128], F32, kind="ExternalInput").ap()
    out = nc.dram_tensor("out", [16, HALF, W], F32, kind="ExternalOutput").ap()
    with tile.TileContext(nc) as tc:
        _emit(tc, xs, vm, out, kh)
    nc.compile()
    _CACHE[key] = nc
    return nc


def _vmats(kv):
    V = np.zeros((8, KDIM, 128), np.float32)
    for r in range(4):
        for b in range(2):
            for m in range(128):
                d = (m - r) % 4
                p_lo = 32 * b + (m - r - d) // 4 + 1
                V[r * 2 + b, p_lo, m] += kv[3 - d]
                if d > 0:
                    V[r * 2 + b, p_lo + 1, m] += kv[7 - d]
    return V


def _slabs(x):
    s = np.zeros((N_CORES, SLAB, W), np.float32)
    for core in range(N_CORES):
        n, half = divmod(core, 2)
        g0 = 512 * half - 4
        s0, s1 = max(0, g0), min(H, g0 + SLAB)
        s[core, s0 - g0: s1 - g0] = x[n, 0, s0:s1]
    return s


def kernel(x, weight):
    x = np.asarray(x, np.float32)
    weight = np.asarray(weight, np.float32)
    assert x.shape == (4, 1, H, W), x.shape
    k2 = weight[0, 0]
    kv = k2[:, 3].astype(np.float64)   # vertical profile (k1)
    kh = k2[3, :].astype(np.float64)   # horizontal profile (k1)

    nc = _build_module(kh)
    V = _vmats(kv)
    slabs = _slabs(x)
    in_maps = [{"xs": slabs[c], "vm": V} for c in range(N_CORES)]
    res = run_bass_kernel_spmd(nc, in_maps, list(range(N_CORES)))

    full = np.empty((4, 16, H, W), np.float32)
    for core in range(N_CORES):
        n, half = divmod(core, 2)
        full[n, :, 512 * half: 512 * half + 512, :] = res.results[core]["out"]
    return full



# revision 3
# speedup vs baseline: 1.0081x; 1.0081x over previous
"""Trainium2 Bass kernel for nn_BayerUpsample4x4.

The reference op: x [4,1,1024,1024] -> 16-channel polyphase 4x bilinear
(tent-filter) upsample, output [4,16,1024,1024].  Each output channel
k=(r,c) is x subsampled at rows==r, cols==c (mod 4), zero-upsampled x4
and convolved with the separable 7x7 tent kernel.

Kernel plan (per core; 8 cores = 4 batches x 2 row-halves):
  Every 128-row x 256-col output block of every channel is ONE bf16
  matmul on TensorE.  For output phase d of channel (r,c) the block is
      w1 * Vt[:, v+delta] + w2 * Vt[:, v+delta+1]
  (Vt = vertical tent interp of the phase-c column lattice).  Both the
  vertical interp and the two-tap horizontal combine are folded into a
  single K=68 contraction: the stationary operand stacks [w1*V34;
  w2*V34] and the moving operand stacks the 34 contributing subrows
  twice, the second copy shifted one subcol (prepared host-side in a
  phase-deinterleaved, zero-halo'd DRAM layout).  Tent weights are
  exact in bf16; only x is rounded (rel err ~1e-3 << 2e-2 gate).

  PSUM->SBUF evacuation is plain fp32 copies balanced across ScalarE
  and VectorE; stores are dense DMAs with 8KB/partition contiguity in
  a custom DRAM layout that the host re-interleaves for free.
"""

import sys
for _p in ("/opt/trn_rl_repo", "/opt/pypackages"):
    if _p not in sys.path:
        sys.path.append(_p)

from contextlib import ExitStack

import numpy as np
import ml_dtypes

import concourse.bass as bass
import concourse.tile as tile
from concourse import bacc, mybir
from concourse.bass_utils import run_bass_kernel_spmd

F32 = mybir.dt.float32
BF16 = mybir.dt.bfloat16
AF = mybir.ActivationFunctionType

N_CORES = 8
H, W = 1024, 1024
HALF = 512               # output rows per core
SLAB = 528               # padded input slab rows per core
KD = 68                  # stacked contraction (34 lo rows + 34 hi rows)
PB = 258                 # halo'd phase-block width (256 + 2 zero borders)
NB = 16                  # (q, r, b) tile combos per core

# (row, col) offset within each 4x4 block for channel k (matches reference)
OFFSETS = [(0, 0), (0, 2), (2, 0), (2, 2),
           (0, 1), (0, 3), (2, 1), (2, 3),
           (1, 0), (1, 2), (3, 0), (3, 2),
           (1, 1), (1, 3), (3, 1), (3, 3)]
K_OF = {rc: k for k, rc in enumerate(OFFSETS)}

# cost-model ns for one [128,512] fp32 PSUM->SBUF copy, for engine balance
_C_ACT = 612.0
_C_DVE = 658.0
_C_GPS = 1200.0

BF = ml_dtypes.bfloat16


def _emit(tc, xs, vv, out, *, use_gps=False, store_gran="qr"):
    """Trace the per-core program.

    xs:  [16, 68, 4, 258] bf16 stacked/deinterleaved input tiles,
         index q*8 + r*2 + b
    vv:  [8, 4, 68, 128] bf16 stacked interp matrices, index (r*2+b, d')
    out: [2, 4, 128, 4, 2, 1024] f32  (q, r, p, c, b, d*256+v)
    """
    nc = tc.nc
    load = {"act": 0.0, "dve": 0.0, "gps": 0.0 if use_gps else 1e15}

    with ExitStack() as ctx:
        vpool = ctx.enter_context(tc.tile_pool(name="vp", bufs=1))
        xpool = ctx.enter_context(tc.tile_pool(name="xp", bufs=4))
        pspool = ctx.enter_context(tc.tile_pool(name="psp", bufs=3,
                                                space="PSUM"))
        opool = ctx.enter_context(tc.tile_pool(name="op", bufs=2))

        vvt = vpool.tile([KD, 8, 4, 128], BF16, tag="vvt")
        nc.sync.dma_start(vvt[:], vv.rearrange("i d p m -> p i d m"))

        for q in range(2):
            for r in range(4):
                ot = opool.tile([128, 4, 2, 1024], F32, tag="ot")
                for b in range(2):
                    idx = q * 8 + r * 2 + b
                    xt = xpool.tile([KD, 4, PB], BF16, tag="xt")
                    nc.sync.dma_start(xt[:], xs[idx])
                    for c in range(4):
                        psA = pspool.tile([128, 512], F32, tag="psA")
                        psB = pspool.tile([128, 512], F32, tag="psB")
                        dst = {0: psA[:, 0:256], 1: psA[:, 256:512],
                               2: psB[:, 0:256], 3: psB[:, 256:512]}
                        for dp in range(4):
                            d = (c + dp) % 4
                            a = 1 if c + dp < 4 else 0
                            nc.tensor.matmul(
                                dst[d],
                                lhsT=vvt[:, r * 2 + b, dp, :],
                                rhs=xt[:, c, a:a + 256],
                                start=True, stop=True,
                            )
                        for halfi, ps in ((0, psA), (1, psB)):
                            dslc = ot[:, c, b, 512 * halfi: 512 * halfi + 512]
                            picks = {"act": load["act"] + _C_ACT,
                                     "dve": load["dve"] + _C_DVE,
                                     "gps": load["gps"] + _C_GPS}
                            eng = min(picks, key=picks.get)
                            load[eng] = picks[eng]
                            if eng == "act":
                                nc.scalar.activation(dslc, ps[:], AF.Copy)
                            elif eng == "dve":
                                nc.vector.tensor_copy(dslc, ps[:])
                            else:
                                nc.gpsimd.tensor_copy(dslc, ps[:])
                if store_gran == "qr":
                    nc.sync.dma_start(out[q, r], ot[:])
                else:
                    for c in range(4):
                        nc.sync.dma_start(out[q, r, :, c], ot[:, c])


_CACHE = {}


def _build_module(key):
    if key in _CACHE:
        return _CACHE[key]
    nc = bacc.Bacc("TRN2", target_bir_lowering=False, debug=False)
    xs = nc.dram_tensor("xs", [NB, KD, 4, PB], BF16, kind="ExternalInput").ap()
    vv = nc.dram_tensor("vv", [8, 4, KD, 128], BF16,
                        kind="ExternalInput").ap()
    out = nc.dram_tensor("out", [2, 4, 128, 4, 2, 1024], F32,
                         kind="ExternalOutput").ap()
    with tile.TileContext(nc) as tc:
        _emit(tc, xs, vv, out)
    nc.compile()
    _CACHE[key] = nc
    return nc


def _vmats(kv):
    """[8, 68, 128] f32 vertical interp matrices, index r*2+b (as before)."""
    V = np.zeros((8, KD, 128), np.float64)
    for r in range(4):
        for b in range(2):
            for m in range(128):
                d = (m - r) % 4
                p_lo = 32 * b + (m - r - d) // 4 + 1
                V[r * 2 + b, p_lo, m] += kv[3 - d]
                if d > 0:
                    V[r * 2 + b, p_lo + 1, m] += kv[7 - d]
    return V


def _vv_mats(kv, kh):
    """[8, 4, 68, 128] stacked matrices: rows 0-33 = w1*V34, 34-67 = w2*V34.

    V34 = V[r*2+b][32b : 32b+34]; (w1, w2) for horizontal phase offset d':
    (1,0), (.75,.25), (.5,.5), (.25,.75).
    """
    V = _vmats(kv)
    VV = np.zeros((8, 4, KD, 128), np.float64)
    for r in range(4):
        for b in range(2):
            v34 = V[r * 2 + b, 32 * b: 32 * b + 34]
            for dp in range(4):
                w1 = float(kh[3 - dp])            # 1, .75, .5, .25
                w2 = float(kh[3 + 4 - dp]) if dp > 0 else 0.0
                VV[r * 2 + b, dp, 0:34] = w1 * v34
                VV[r * 2 + b, dp, 34:68] = w2 * v34
    return VV


def _slabs(x):
    s = np.zeros((N_CORES, SLAB, W), np.float32)
    for core in range(N_CORES):
        n, half = divmod(core, 2)
        g0 = 512 * half - 4
        s0, s1 = max(0, g0), min(H, g0 + SLAB)
        s[core, s0 - g0: s1 - g0] = x[n, 0, s0:s1]
    return s


def _xtiles(slab):
    """slab [528, 1024] f32 -> [16, 68, 4, 258] bf16 stacked tiles."""
    xt = np.zeros((2, 4, 2, KD, 4, PB), np.float32)
    for q in range(2):
        for r in range(4):
            for b in range(2):
                i0 = 64 * q + 32 * b
                rows = slab[4 * i0 + r: 4 * i0 + r + 4 * 34: 4]  # [34, 1024]
                bs = rows.reshape(34, 256, 4).transpose(0, 2, 1)  # [34,4,256]
                xt[q, r, b, 0:34, :, 1:257] = bs
                xt[q, r, b, 34:68, :, 0:256] = bs
    return xt.reshape(NB, KD, 4, PB).astype(BF)


_PERM = [rr * 4 + cc for (rr, cc) in OFFSETS]   # k -> flat (r, c) index


def _unshuffle(res):
    """Device out [2,4,128,4,2,4,256] (q,r,p,c,b,d,v) -> [16, 512, 1024]."""
    a = res.reshape(2, 4, 128, 4, 2, 4, 256)
    # target [k(r,c), row = 256q+128b+p, col = 4v+d]
    a = a.transpose(1, 3, 0, 4, 2, 6, 5)      # [r, c, q, b, p, v, d]
    a = np.ascontiguousarray(a).reshape(16, 512, 1024)
    return a[_PERM]


def kernel(x, weight):
    x = np.asarray(x, np.float32)
    weight = np.asarray(weight, np.float32)
    assert x.shape == (4, 1, H, W), x.shape
    k2 = weight[0, 0]
    kv = k2[:, 3].astype(np.float64)   # vertical profile
    kh = k2[3, :].astype(np.float64)   # horizontal profile

    nc = _build_module(tuple(np.asarray(k2, np.float64).ravel().tolist()))
    VV = _vv_mats(kv, kh).astype(BF)
    slabs = _slabs(x)
    in_maps = [{"xs": _xtiles(slabs[c]), "vv": VV} for c in range(N_CORES)]
    res = run_bass_kernel_spmd(nc, in_maps, list(range(N_CORES)))

    full = np.empty((4, 16, H, W), np.float32)
    for core in range(N_CORES):
        n, half = divmod(core, 2)
        full[n, :, 512 * half: 512 * half + 512, :] = \
            _unshuffle(np.asarray(res.results[core]["out"], np.float32))
    return full


# revision 14
# speedup vs baseline: 1.3007x; 1.2903x over previous
"""Trainium2 Bass kernel for nn_BayerUpsample4x4.

The reference op: x [4,1,1024,1024] -> 16-channel polyphase 4x bilinear
(tent-filter) upsample, output [4,16,1024,1024].  Each output channel
k=(r,c) is x subsampled at rows==r, cols==c (mod 4), zero-upsampled x4
and convolved with the separable 7x7 tent kernel.

Kernel plan (per core; 8 cores = 4 batches x 2 row-halves):
  Every 128-row x 256-col output block of every channel is ONE bf16
  matmul on TensorE.  For output phase d of channel (r,c) the block is
      w1 * Vt[:, v+delta] + w2 * Vt[:, v+delta+1]
  (Vt = vertical tent interp of the phase-c column lattice).  Both the
  vertical interp and the two-tap horizontal combine are folded into a
  single K=68 contraction: the stationary operand stacks [w1*V34;
  w2*V34] and the moving operand stacks the 34 contributing subrows
  twice, the second copy shifted one subcol (prepared host-side in a
  phase-deinterleaved, zero-halo'd DRAM layout).  Tent weights are
  exact in bf16; only x is rounded (rel err ~1e-3 << 2e-2 gate).

  PSUM->SBUF evacuation is plain fp32 copies balanced across ScalarE
  and VectorE; stores are dense DMAs with 8KB/partition contiguity in
  a custom DRAM layout that the host re-interleaves for free.
"""

import sys
for _p in ("/opt/trn_rl_repo", "/opt/pypackages"):
    if _p not in sys.path:
        sys.path.append(_p)

from contextlib import ExitStack

import numpy as np
import ml_dtypes

import concourse.bass as bass
import concourse.tile as tile
from concourse import bacc, mybir
from concourse.bass_utils import run_bass_kernel_spmd

F32 = mybir.dt.float32
BF16 = mybir.dt.bfloat16
AF = mybir.ActivationFunctionType

N_CORES = 8
H, W = 1024, 1024
HALF = 512               # output rows per core
SLAB = 528               # padded input slab rows per core
KD = 68                  # stacked contraction (34 lo rows + 34 hi rows)
PB = 258                 # halo'd phase-block width (256 + 2 zero borders)
NB = 16                  # (q, r, b) tile combos per core

# (row, col) offset within each 4x4 block for channel k (matches reference)
OFFSETS = [(0, 0), (0, 2), (2, 0), (2, 2),
           (0, 1), (0, 3), (2, 1), (2, 3),
           (1, 0), (1, 2), (3, 0), (3, 2),
           (1, 1), (1, 3), (3, 1), (3, 3)]
K_OF = {rc: k for k, rc in enumerate(OFFSETS)}

# cost-model ns for one [128,512] fp32 PSUM->SBUF copy, for engine balance
_C_ACT = 612.0
_C_DVE = 658.0
_C_GPS = 1200.0

BF = ml_dtypes.bfloat16


def _emit_loads(ctx, tc, xs, vv):
    """Load the (loop-invariant) inputs into SBUF once: the stacked
    interp matrices and the whole stacked input (33KB/partition).
    `ctx` is an ExitStack that must outlive every _emit_body call."""
    nc = tc.nc
    vpool = ctx.enter_context(tc.tile_pool(name="vp", bufs=1))
    vvt = vpool.tile([KD, 8, 4, 128], BF16, tag="vvt")
    nc.sync.dma_start(vvt[:], vv.rearrange("i d p m -> p i d m"))
    xall = vpool.tile([KD, NB, 4, PB], BF16, tag="xall")
    nc.sync.dma_start(xall[:], xs.rearrange("i p s h -> p i s h"))
    return vvt, xall


def _emit_body(tc, vvt, xall, out):
    """One full pass: 256 matmuls, 128 evac copies, 8 stores of 4MB.

    out: [2, 4, 128, 4, 2, 1024] f32  (q, r, p, c, b, d*256+v)
    """
    nc = tc.nc
    with ExitStack() as ctx:
        pspool = ctx.enter_context(tc.tile_pool(name="psp", bufs=3,
                                                space="PSUM"))
        opool = ctx.enter_context(tc.tile_pool(name="op", bufs=3))

        for q in range(2):
            for r in range(4):
                ot = opool.tile([128, 4, 2, 1024], F32, tag="ot")
                for b in range(2):
                    idx = q * 8 + r * 2 + b
                    for c in range(4):
                        psA = pspool.tile([128, 512], F32, tag="psA")
                        psB = pspool.tile([128, 512], F32, tag="psB")
                        dst = {0: psA[:, 0:256], 1: psA[:, 256:512],
                               2: psB[:, 0:256], 3: psB[:, 256:512]}
                        for dp in range(4):
                            d = (c + dp) % 4
                            a = 1 if c + dp < 4 else 0
                            nc.tensor.matmul(
                                dst[d],
                                lhsT=vvt[:, r * 2 + b, dp, :],
                                rhs=xall[:, idx, c, a:a + 256],
                                start=True, stop=True,
                            )
                        nc.scalar.activation(ot[:, c, b, 0:512], psA[:],
                                             AF.Copy)
                        nc.vector.tensor_copy(ot[:, c, b, 512:1024], psB[:])
                nc.sync.dma_start(out[q, r], ot[:])


def _emit(tc, xs, vv, out):
    with ExitStack() as ctx:
        vvt, xall = _emit_loads(ctx, tc, xs, vv)
        _emit_body(tc, vvt, xall, out)


_CACHE = {}


def _build_module(key):
    if key in _CACHE:
        return _CACHE[key]
    nc = bacc.Bacc("TRN2", target_bir_lowering=False, debug=False)
    xs = nc.dram_tensor("xs", [NB, KD, 4, PB], BF16, kind="ExternalInput").ap()
    vv = nc.dram_tensor("vv", [8, 4, KD, 128], BF16,
                        kind="ExternalInput").ap()
    out = nc.dram_tensor("out", [2, 4, 128, 4, 2, 1024], F32,
                         kind="ExternalOutput").ap()
    with tile.TileContext(nc) as tc:
        _emit(tc, xs, vv, out)
    nc.compile()
    _CACHE[key] = nc
    return nc


def _vmats(kv):
    """[8, 68, 128] f32 vertical interp matrices, index r*2+b (as before)."""
    V = np.zeros((8, KD, 128), np.float64)
    for r in range(4):
        for b in range(2):
            for m in range(128):
                d = (m - r) % 4
                p_lo = 32 * b + (m - r - d) // 4 + 1
                V[r * 2 + b, p_lo, m] += kv[3 - d]
                if d > 0:
                    V[r * 2 + b, p_lo + 1, m] += kv[7 - d]
    return V


def _vv_mats(kv, kh):
    """[8, 4, 68, 128] stacked matrices: rows 0-33 = w1*V34, 34-67 = w2*V34.

    V34 = V[r*2+b][32b : 32b+34]; (w1, w2) for horizontal phase offset d':
    (1,0), (.75,.25), (.5,.5), (.25,.75).
    """
    V = _vmats(kv)
    VV = np.zeros((8, 4, KD, 128), np.float64)
    for r in range(4):
        for b in range(2):
            v34 = V[r * 2 + b, 32 * b: 32 * b + 34]
            for dp in range(4):
                w1 = float(kh[3 - dp])            # 1, .75, .5, .25
                w2 = float(kh[3 + 4 - dp]) if dp > 0 else 0.0
                VV[r * 2 + b, dp, 0:34] = w1 * v34
                VV[r * 2 + b, dp, 34:68] = w2 * v34
    return VV


def _slabs(x):
    s = np.zeros((N_CORES, SLAB, W), np.float32)
    for core in range(N_CORES):
        n, half = divmod(core, 2)
        g0 = 512 * half - 4
        s0, s1 = max(0, g0), min(H, g0 + SLAB)
        s[core, s0 - g0: s1 - g0] = x[n, 0, s0:s1]
    return s


def _xtiles(slab):
    """slab [528, 1024] f32 -> [16, 68, 4, 258] bf16 stacked tiles
    (rows 0-33 = contributing subrows; 34-67 = same, one subcol left)."""
    xt = np.zeros((2, 4, 2, KD, 4, PB), np.float32)
    for q in range(2):
        for r in range(4):
            for b in range(2):
                i0 = 64 * q + 32 * b
                rows = slab[4 * i0 + r: 4 * i0 + r + 4 * 34: 4]  # [34, 1024]
                bs = rows.reshape(34, 256, 4).transpose(0, 2, 1)  # [34,4,256]
                xt[q, r, b, 0:34, :, 1:257] = bs
                xt[q, r, b, 34:68, :, 0:256] = bs
    return xt.reshape(NB, KD, 4, PB).astype(BF)


_PERM = [rr * 4 + cc for (rr, cc) in OFFSETS]   # k -> flat (r, c) index


def _unshuffle(res):
    """Device out [2,4,128,4,2,4,256] (q,r,p,c,b,d,v) -> [16, 512, 1024]."""
    a = res.reshape(2, 4, 128, 4, 2, 4, 256)
    # target [k(r,c), row = 256q+128b+p, col = 4v+d]
    a = a.transpose(1, 3, 0, 4, 2, 6, 5)      # [r, c, q, b, p, v, d]
    a = np.ascontiguousarray(a).reshape(16, 512, 1024)
    return a[_PERM]


def kernel(x, weight):
    x = np.asarray(x, np.float32)
    weight = np.asarray(weight, np.float32)
    assert x.shape == (4, 1, H, W), x.shape
    k2 = weight[0, 0]
    kv = k2[:, 3].astype(np.float64)   # vertical profile
    kh = k2[3, :].astype(np.float64)   # horizontal profile

    nc = _build_module(tuple(np.asarray(k2, np.float64).ravel().tolist()))
    VV = _vv_mats(kv, kh).astype(BF)
    slabs = _slabs(x)
    in_maps = [{"xs": _xtiles(slabs[c]), "vv": VV} for c in range(N_CORES)]
    res = run_bass_kernel_spmd(nc, in_maps, list(range(N_CORES)))

    full = np.empty((4, 16, H, W), np.float32)
    for core in range(N_CORES):
        n, half = divmod(core, 2)
        full[n, :, 512 * half: 512 * half + 512, :] = \
            _unshuffle(np.asarray(res.results[core]["out"], np.float32))
    return full


# revision 16
# speedup vs baseline: 1.3339x; 1.0255x over previous
"""Trainium2 Bass kernel for nn_BayerUpsample4x4.

The reference op: x [4,1,1024,1024] -> 16-channel polyphase 4x bilinear
(tent-filter) upsample, output [4,16,1024,1024].  Each output channel
k=(r,c) is x subsampled at rows==r, cols==c (mod 4), zero-upsampled x4
and convolved with the separable 7x7 tent kernel.

Kernel plan (per core; 8 cores = 4 batches x 2 row-halves):
  Every 128-row x 256-col output block of every channel is ONE bf16
  matmul on TensorE.  For output phase d of channel (r,c) the block is
      w1 * Vt[:, v+delta] + w2 * Vt[:, v+delta+1]
  (Vt = vertical tent interp of the phase-c column lattice).  Both the
  vertical interp and the two-tap horizontal combine are folded into a
  single K=68 contraction: the stationary operand stacks [w1*V34;
  w2*V34] and the moving operand stacks the 34 contributing subrows
  twice, the second copy shifted one subcol (prepared host-side in a
  phase-deinterleaved, zero-halo'd DRAM layout).  Tent weights are
  exact in bf16; only x is rounded (rel err ~3e-3 << 2e-2 gate).

  PSUM->SBUF evacuation is plain fp32 copies split ScalarE/VectorE;
  stores are eight 4MB DMAs with 32KB/partition contiguity in a custom
  DRAM layout that the host re-interleaves for free.

Measured decomposition (per core, robust For_i-delta method):
  stores-only floor 97.7us (333 GB/s/core); +engine activity ~8us
  (platform-level interference, invariant to structure); any HBM loads
  mixed into the store stream cost ~10x their data time, so the whole
  input (33KB/partition) is loaded once up front and reused.
"""

import sys
for _p in ("/opt/trn_rl_repo", "/opt/pypackages"):
    if _p not in sys.path:
        sys.path.append(_p)

from contextlib import ExitStack

import numpy as np
import ml_dtypes

import concourse.bass as bass
import concourse.tile as tile
from concourse import bacc, mybir
from concourse.bass_utils import run_bass_kernel_spmd

F32 = mybir.dt.float32
BF16 = mybir.dt.bfloat16
AF = mybir.ActivationFunctionType

N_CORES = 8
H, W = 1024, 1024
HALF = 512               # output rows per core
SLAB = 528               # padded input slab rows per core
KD = 68                  # stacked contraction (34 lo rows + 34 hi rows)
PB = 258                 # halo'd phase-block width (256 + 2 zero borders)
NB = 16                  # (q, r, b) tile combos per core

# (row, col) offset within each 4x4 block for channel k (matches reference)
OFFSETS = [(0, 0), (0, 2), (2, 0), (2, 2),
           (0, 1), (0, 3), (2, 1), (2, 3),
           (1, 0), (1, 2), (3, 0), (3, 2),
           (1, 1), (1, 3), (3, 1), (3, 3)]
K_OF = {rc: k for k, rc in enumerate(OFFSETS)}

BF = ml_dtypes.bfloat16


def _emit_loads(ctx, tc, xs, vv):
    """Load the (loop-invariant) inputs into SBUF once: the stacked
    interp matrices and the whole stacked input (33KB/partition).
    `ctx` is an ExitStack that must outlive every _emit_body call."""
    nc = tc.nc
    vpool = ctx.enter_context(tc.tile_pool(name="vp", bufs=1))
    vvt = vpool.tile([KD, 8, 4, 128], BF16, tag="vvt")
    nc.sync.dma_start(vvt[:], vv.rearrange("i d p m -> p i d m"))
    xall = vpool.tile([KD, NB, 4, PB], BF16, tag="xall")
    nc.sync.dma_start(xall[:], xs.rearrange("i p s h -> p i s h"))
    return vvt, xall


def _emit_body(tc, vvt, xall, out):
    """One full pass: 256 matmuls, 128 evac copies, 8 stores of 4MB.

    out: [2, 4, 128, 4, 2, 1024] f32  (q, r, p, c, b, d*256+v)
    """
    nc = tc.nc
    with ExitStack() as ctx:
        pspool = ctx.enter_context(tc.tile_pool(name="psp", bufs=3,
                                                space="PSUM"))
        opool = ctx.enter_context(tc.tile_pool(name="op", bufs=3))

        for q in range(2):
            for r in range(4):
                ot = opool.tile([128, 4, 2, 1024], F32, tag="ot")
                for b in range(2):
                    idx = q * 8 + r * 2 + b
                    for c in range(4):
                        psA = pspool.tile([128, 512], F32, tag="psA")
                        psB = pspool.tile([128, 512], F32, tag="psB")
                        dst = {0: psA[:, 0:256], 1: psA[:, 256:512],
                               2: psB[:, 0:256], 3: psB[:, 256:512]}
                        for dp in range(4):
                            d = (c + dp) % 4
                            a = 1 if c + dp < 4 else 0
                            nc.tensor.matmul(
                                dst[d],
                                lhsT=vvt[:, r * 2 + b, dp, :],
                                rhs=xall[:, idx, c, a:a + 256],
                                start=True, stop=True,
                            )
                        nc.scalar.activation(ot[:, c, b, 0:512], psA[:],
                                             AF.Copy)
                        nc.vector.tensor_copy(ot[:, c, b, 512:1024], psB[:])
                nc.sync.dma_start(out[q, r], ot[:])


def _emit(tc, xs, vv, out):
    with ExitStack() as ctx:
        vvt, xall = _emit_loads(ctx, tc, xs, vv)
        _emit_body(tc, vvt, xall, out)


_CACHE = {}


def _build_module(key):
    if key in _CACHE:
        return _CACHE[key]
    nc = bacc.Bacc("TRN2", target_bir_lowering=False, debug=False)
    xs = nc.dram_tensor("xs", [NB, KD, 4, PB], BF16, kind="ExternalInput").ap()
    vv = nc.dram_tensor("vv", [8, 4, KD, 128], BF16,
                        kind="ExternalInput").ap()
    out = nc.dram_tensor("out", [2, 4, 128, 4, 2, 1024], F32,
                         kind="ExternalOutput").ap()
    with tile.TileContext(nc) as tc:
        _emit(tc, xs, vv, out)
    nc.compile()
    _CACHE[key] = nc
    return nc


def _vmats(kv):
    """[8, 68, 128] f32 vertical interp matrices, index r*2+b (as before)."""
    V = np.zeros((8, KD, 128), np.float64)
    for r in range(4):
        for b in range(2):
            for m in range(128):
                d = (m - r) % 4
                p_lo = 32 * b + (m - r - d) // 4 + 1
                V[r * 2 + b, p_lo, m] += kv[3 - d]
                if d > 0:
                    V[r * 2 + b, p_lo + 1, m] += kv[7 - d]
    return V


def _vv_mats(kv, kh):
    """[8, 4, 68, 128] stacked matrices: rows 0-33 = w1*V34, 34-67 = w2*V34.

    V34 = V[r*2+b][32b : 32b+34]; (w1, w2) for horizontal phase offset d':
    (1,0), (.75,.25), (.5,.5), (.25,.75).
    """
    V = _vmats(kv)
    VV = np.zeros((8, 4, KD, 128), np.float64)
    for r in range(4):
        for b in range(2):
            v34 = V[r * 2 + b, 32 * b: 32 * b + 34]
            for dp in range(4):
                w1 = float(kh[3 - dp])            # 1, .75, .5, .25
                w2 = float(kh[3 + 4 - dp]) if dp > 0 else 0.0
                VV[r * 2 + b, dp, 0:34] = w1 * v34
                VV[r * 2 + b, dp, 34:68] = w2 * v34
    return VV


def _slabs(x):
    s = np.zeros((N_CORES, SLAB, W), np.float32)
    for core in range(N_CORES):
        n, half = divmod(core, 2)
        g0 = 512 * half - 4
        s0, s1 = max(0, g0), min(H, g0 + SLAB)
        s[core, s0 - g0: s1 - g0] = x[n, 0, s0:s1]
    return s


def _xtiles(slab):
    """slab [528, 1024] f32 -> [16, 68, 4, 258] bf16 stacked tiles
    (rows 0-33 = contributing subrows; 34-67 = same, one subcol left)."""
    xt = np.zeros((2, 4, 2, KD, 4, PB), np.float32)
    for q in range(2):
        for r in range(4):
            for b in range(2):
                i0 = 64 * q + 32 * b
                rows = slab[4 * i0 + r: 4 * i0 + r + 4 * 34: 4]  # [34, 1024]
                bs = rows.reshape(34, 256, 4).transpose(0, 2, 1)  # [34,4,256]
                xt[q, r, b, 0:34, :, 1:257] = bs
                xt[q, r, b, 34:68, :, 0:256] = bs
    return xt.reshape(NB, KD, 4, PB).astype(BF)


_PERM = [rr * 4 + cc for (rr, cc) in OFFSETS]   # k -> flat (r, c) index


def _unshuffle(res):
    """Device out [2,4,128,4,2,4,256] (q,r,p,c,b,d,v) -> [16, 512, 1024]."""
    a = res.reshape(2, 4, 128, 4, 2, 4, 256)
    # target [k(r,c), row = 256q+128b+p, col = 4v+d]
    a = a.transpose(1, 3, 0, 4, 2, 6, 5)      # [r, c, q, b, p, v, d]
    a = np.ascontiguousarray(a).reshape(16, 512, 1024)
    return a[_PERM]


def kernel(x, weight):
    x = np.asarray(x, np.float32)
    weight = np.asarray(weight, np.float32)
    assert x.shape == (4, 1, H, W), x.shape
    k2 = weight[0, 0]
    kv = k2[:, 3].astype(np.float64)   # vertical profile
    kh = k2[3, :].astype(np.float64)   # horizontal profile

    nc = _build_module(tuple(np.asarray(k2, np.float64).ravel().tolist()))
    VV = _vv_mats(kv, kh).astype(BF)
    slabs = _slabs(x)
    in_maps = [{"xs": _xtiles(slabs[c]), "vv": VV} for c in range(N_CORES)]
    res = run_bass_kernel_spmd(nc, in_maps, list(range(N_CORES)))

    full = np.empty((4, 16, H, W), np.float32)
    for core in range(N_CORES):
        n, half = divmod(core, 2)
        full[n, :, 512 * half: 512 * half + 512, :] = \
            _unshuffle(np.asarray(res.results[core]["out"], np.float32))
    return full


# revision 18
# speedup vs baseline: 2.1847x; 1.6378x over previous
"""Trainium2 Bass kernel for nn_BayerUpsample4x4.

The reference op: x [4,1,1024,1024] -> 16-channel polyphase 4x bilinear
(tent-filter) upsample, output [4,16,1024,1024].  Each output channel
k=(r,c) is x subsampled at rows==r, cols==c (mod 4), zero-upsampled x4
and convolved with the separable 7x7 tent kernel.

Kernel plan (per core; 8 cores = 4 batches x 2 row-halves):
  Every 128-row x 256-col output block of every channel is ONE bf16
  matmul on TensorE.  For output phase d of channel (r,c) the block is
      w1 * Vt[:, v+delta] + w2 * Vt[:, v+delta+1]
  (Vt = vertical tent interp of the phase-c column lattice).  Both the
  vertical interp and the two-tap horizontal combine are folded into a
  single K=68 contraction: the stationary operand stacks [w1*V34;
  w2*V34] and the moving operand stacks the 34 contributing subrows
  twice, the second copy shifted one subcol (prepared host-side in a
  phase-deinterleaved, zero-halo'd DRAM layout).  Tent weights are
  exact in bf16; only x is rounded (rel err ~3e-3 << 2e-2 gate).

  PSUM->SBUF evacuation is plain fp32 copies split ScalarE/VectorE;
  stores are eight 4MB DMAs with 32KB/partition contiguity in a custom
  DRAM layout that the host re-interleaves for free.

Measured decomposition (per core, robust For_i-delta method):
  stores-only floor 97.7us (333 GB/s/core); +engine activity ~8us
  (platform-level interference, invariant to structure); any HBM loads
  mixed into the store stream cost ~10x their data time, so the whole
  input (33KB/partition) is loaded once up front and reused.
"""

import sys
for _p in ("/opt/trn_rl_repo", "/opt/pypackages"):
    if _p not in sys.path:
        sys.path.append(_p)

from contextlib import ExitStack

import numpy as np
import ml_dtypes

import concourse.bass as bass
import concourse.tile as tile
from concourse import bacc, mybir
from concourse.bass_utils import run_bass_kernel_spmd

F32 = mybir.dt.float32
BF16 = mybir.dt.bfloat16
AF = mybir.ActivationFunctionType

N_CORES = 8
H, W = 1024, 1024
HALF = 512               # output rows per core
SLAB = 528               # padded input slab rows per core
KD = 68                  # stacked contraction (34 lo rows + 34 hi rows)
PB = 258                 # halo'd phase-block width (256 + 2 zero borders)
NB = 16                  # (q, r, b) tile combos per core

# (row, col) offset within each 4x4 block for channel k (matches reference)
OFFSETS = [(0, 0), (0, 2), (2, 0), (2, 2),
           (0, 1), (0, 3), (2, 1), (2, 3),
           (1, 0), (1, 2), (3, 0), (3, 2),
           (1, 1), (1, 3), (3, 1), (3, 3)]
K_OF = {rc: k for k, rc in enumerate(OFFSETS)}

BF = ml_dtypes.bfloat16


def _emit_loads(ctx, tc, xs, vv):
    """Load the (loop-invariant) inputs into SBUF once: the stacked
    interp matrices and the whole stacked input (33KB/partition).
    `ctx` is an ExitStack that must outlive every _emit_body call."""
    nc = tc.nc
    vpool = ctx.enter_context(tc.tile_pool(name="vp", bufs=1))
    vvt = vpool.tile([KD, 8, 4, 128], BF16, tag="vvt")
    nc.sync.dma_start(vvt[:], vv.rearrange("i d p m -> p i d m"))
    xall = vpool.tile([KD, NB, 4, PB], BF16, tag="xall")
    nc.sync.dma_start(xall[:], xs.rearrange("i p s h -> p i s h"))
    return vvt, xall


def _emit_body(tc, vvt, xall, out):
    """One full pass: 256 matmuls, 128 evac copies, 8 stores of 2MB.

    out: [2, 4, 128, 4, 2, 1024] bf16  (q, r, p, c, b, d*256+v)

    The evac copies convert fp32 PSUM -> bf16 SBUF on their write path
    (free), halving the store stream to 16MB/core; the host upconverts
    to fp32 during the unshuffle.  Output rounding adds ~1e-3 to the
    scale-relative error (4.1e-3 total vs the 2e-2 gate).
    """
    nc = tc.nc
    with ExitStack() as ctx:
        pspool = ctx.enter_context(tc.tile_pool(name="psp", bufs=3,
                                                space="PSUM"))
        opool = ctx.enter_context(tc.tile_pool(name="op", bufs=3))

        for q in range(2):
            for r in range(4):
                ot = opool.tile([128, 4, 2, 1024], BF16, tag="ot")
                for b in range(2):
                    idx = q * 8 + r * 2 + b
                    for c in range(4):
                        psA = pspool.tile([128, 512], F32, tag="psA")
                        psB = pspool.tile([128, 512], F32, tag="psB")
                        dst = {0: psA[:, 0:256], 1: psA[:, 256:512],
                               2: psB[:, 0:256], 3: psB[:, 256:512]}
                        for dp in range(4):
                            d = (c + dp) % 4
                            a = 1 if c + dp < 4 else 0
                            nc.tensor.matmul(
                                dst[d],
                                lhsT=vvt[:, r * 2 + b, dp, :],
                                rhs=xall[:, idx, c, a:a + 256],
                                start=True, stop=True,
                            )
                        nc.scalar.activation(ot[:, c, b, 0:512], psA[:],
                                             AF.Copy)
                        nc.vector.tensor_copy(ot[:, c, b, 512:1024], psB[:])
                nc.sync.dma_start(out[q, r], ot[:])


def _emit(tc, xs, vv, out):
    with ExitStack() as ctx:
        vvt, xall = _emit_loads(ctx, tc, xs, vv)
        _emit_body(tc, vvt, xall, out)


_CACHE = {}


def _build_module(key):
    if key in _CACHE:
        return _CACHE[key]
    nc = bacc.Bacc("TRN2", target_bir_lowering=False, debug=False)
    xs = nc.dram_tensor("xs", [NB, KD, 4, PB], BF16, kind="ExternalInput").ap()
    vv = nc.dram_tensor("vv", [8, 4, KD, 128], BF16,
                        kind="ExternalInput").ap()
    out = nc.dram_tensor("out", [2, 4, 128, 4, 2, 1024], BF16,
                         kind="ExternalOutput").ap()
    with tile.TileContext(nc) as tc:
        _emit(tc, xs, vv, out)
    nc.compile()
    _CACHE[key] = nc
    return nc


def _vmats(kv):
    """[8, 68, 128] f32 vertical interp matrices, index r*2+b (as before)."""
    V = np.zeros((8, KD, 128), np.float64)
    for r in range(4):
        for b in range(2):
            for m in range(128):
                d = (m - r) % 4
                p_lo = 32 * b + (m - r - d) // 4 + 1
                V[r * 2 + b, p_lo, m] += kv[3 - d]
                if d > 0:
                    V[r * 2 + b, p_lo + 1, m] += kv[7 - d]
    return V


def _vv_mats(kv, kh):
    """[8, 4, 68, 128] stacked matrices: rows 0-33 = w1*V34, 34-67 = w2*V34.

    V34 = V[r*2+b][32b : 32b+34]; (w1, w2) for horizontal phase offset d':
    (1,0), (.75,.25), (.5,.5), (.25,.75).
    """
    V = _vmats(kv)
    VV = np.zeros((8, 4, KD, 128), np.float64)
    for r in range(4):
        for b in range(2):
            v34 = V[r * 2 + b, 32 * b: 32 * b + 34]
            for dp in range(4):
                w1 = float(kh[3 - dp])            # 1, .75, .5, .25
                w2 = float(kh[3 + 4 - dp]) if dp > 0 else 0.0
                VV[r * 2 + b, dp, 0:34] = w1 * v34
                VV[r * 2 + b, dp, 34:68] = w2 * v34
    return VV


def _slabs(x):
    s = np.zeros((N_CORES, SLAB, W), np.float32)
    for core in range(N_CORES):
        n, half = divmod(core, 2)
        g0 = 512 * half - 4
        s0, s1 = max(0, g0), min(H, g0 + SLAB)
        s[core, s0 - g0: s1 - g0] = x[n, 0, s0:s1]
    return s


def _xtiles(slab):
    """slab [528, 1024] f32 -> [16, 68, 4, 258] bf16 stacked tiles
    (rows 0-33 = contributing subrows; 34-67 = same, one subcol left)."""
    xt = np.zeros((2, 4, 2, KD, 4, PB), np.float32)
    for q in range(2):
        for r in range(4):
            for b in range(2):
                i0 = 64 * q + 32 * b
                rows = slab[4 * i0 + r: 4 * i0 + r + 4 * 34: 4]  # [34, 1024]
                bs = rows.reshape(34, 256, 4).transpose(0, 2, 1)  # [34,4,256]
                xt[q, r, b, 0:34, :, 1:257] = bs
                xt[q, r, b, 34:68, :, 0:256] = bs
    return xt.reshape(NB, KD, 4, PB).astype(BF)


_PERM = [rr * 4 + cc for (rr, cc) in OFFSETS]   # k -> flat (r, c) index


def _unshuffle(res):
    """Device out [2,4,128,4,2,4,256] (q,r,p,c,b,d,v) -> [16, 512, 1024]."""
    a = res.reshape(2, 4, 128, 4, 2, 4, 256)
    # target [k(r,c), row = 256q+128b+p, col = 4v+d]
    a = a.transpose(1, 3, 0, 4, 2, 6, 5)      # [r, c, q, b, p, v, d]
    a = np.ascontiguousarray(a).reshape(16, 512, 1024)
    return a[_PERM]


def kernel(x, weight):
    x = np.asarray(x, np.float32)
    weight = np.asarray(weight, np.float32)
    assert x.shape == (4, 1, H, W), x.shape
    k2 = weight[0, 0]
    kv = k2[:, 3].astype(np.float64)   # vertical profile
    kh = k2[3, :].astype(np.float64)   # horizontal profile

    nc = _build_module(tuple(np.asarray(k2, np.float64).ravel().tolist()))
    VV = _vv_mats(kv, kh).astype(BF)
    slabs = _slabs(x)
    in_maps = [{"xs": _xtiles(slabs[c]), "vv": VV} for c in range(N_CORES)]
    res = run_bass_kernel_spmd(nc, in_maps, list(range(N_CORES)))

    full = np.empty((4, 16, H, W), np.float32)
    for core in range(N_CORES):
        n, half = divmod(core, 2)
        full[n, :, 512 * half: 512 * half + 512, :] = \
            _unshuffle(np.asarray(res.results[core]["out"], np.float32))
    return full


# revision 22
# speedup vs baseline: 2.4109x; 1.1035x over previous
"""Trainium2 Bass kernel for nn_BayerUpsample4x4.

The reference op: x [4,1,1024,1024] -> 16-channel polyphase 4x bilinear
(tent-filter) upsample, output [4,16,1024,1024].  Each output channel
k=(r,c) is x subsampled at rows==r, cols==c (mod 4), zero-upsampled x4
and convolved with the separable 7x7 tent kernel.

Kernel plan (per core; 8 cores = 4 batches x 2 row-halves):
  Every 128-row x 256-col output block of every channel is ONE bf16
  matmul on TensorE.  For output phase d of channel (r,c) the block is
      w1 * Vt[:, v+delta] + w2 * Vt[:, v+delta+1]
  (Vt = vertical tent interp of the phase-c column lattice).  Both the
  vertical interp and the two-tap horizontal combine are folded into a
  single K=68 contraction: the stationary operand stacks [w1*V34;
  w2*V34] and the moving operand stacks the 34 contributing subrows
  twice, the second copy shifted one subcol (prepared host-side in a
  phase-deinterleaved, zero-halo'd DRAM layout).  Tent weights are
  exact in bf16; only x is rounded (rel err ~3e-3 << 2e-2 gate).

  PSUM->SBUF evacuation is plain fp32 copies split ScalarE/VectorE;
  stores are eight 4MB DMAs with 32KB/partition contiguity in a custom
  DRAM layout that the host re-interleaves for free.

Measured decomposition (per core, robust For_i-delta method):
  stores-only floor 97.7us (333 GB/s/core); +engine activity ~8us
  (platform-level interference, invariant to structure); any HBM loads
  mixed into the store stream cost ~10x their data time, so the whole
  input (33KB/partition) is loaded once up front and reused.
"""

import sys
for _p in ("/opt/trn_rl_repo", "/opt/pypackages"):
    if _p not in sys.path:
        sys.path.append(_p)

from contextlib import ExitStack

import numpy as np
import ml_dtypes

import concourse.bass as bass
import concourse.tile as tile
from concourse import bacc, mybir
from concourse.bass_utils import run_bass_kernel_spmd

F32 = mybir.dt.float32
BF16 = mybir.dt.bfloat16
AF = mybir.ActivationFunctionType

N_CORES = 8
H, W = 1024, 1024
HALF = 512               # output rows per core
SLAB = 528               # padded input slab rows per core
KD = 68                  # stacked contraction (34 lo rows + 34 hi rows)
KDP = 128                # K padded to 128: NumWeights==128 enables the
                         # fast-weight-load path (measured 217 -> 117 ns/MM)
PB = 258                 # halo'd phase-block width (256 + 2 zero borders)
NB = 16                  # (q, r, b) tile combos per core

# (row, col) offset within each 4x4 block for channel k (matches reference)
OFFSETS = [(0, 0), (0, 2), (2, 0), (2, 2),
           (0, 1), (0, 3), (2, 1), (2, 3),
           (1, 0), (1, 2), (3, 0), (3, 2),
           (1, 1), (1, 3), (3, 1), (3, 3)]
K_OF = {rc: k for k, rc in enumerate(OFFSETS)}

BF = ml_dtypes.bfloat16


def _emit_loads(ctx, tc, xs, vv):
    """Load the (loop-invariant) inputs into SBUF once: the stacked
    interp matrices and the whole stacked input (33KB/partition).
    `ctx` is an ExitStack that must outlive every _emit_body call."""
    nc = tc.nc
    vpool = ctx.enter_context(tc.tile_pool(name="vp", bufs=1))
    vvt = vpool.tile([KDP, 8, 4, 128], BF16, tag="vvt")
    nc.sync.dma_start(vvt[:], vv.rearrange("i d p m -> p i d m"))
    xall = vpool.tile([KDP, NB, 4, PB], BF16, tag="xall")
    nc.vector.memset(xall[64:128], 0.0)   # pad rows: zero x garbage
    nc.sync.dma_start(xall[0:KD], xs.rearrange("i p s h -> p i s h"))
    return vvt, xall


def _emit_body(tc, vvt, xall, out):
    """One full pass: 256 matmuls, 128 evac copies, 8 stores of 2MB.

    out: [2, 4, 128, 4, 2, 1024] bf16  (q, r, p, c, b, d*256+v)

    The evac copies convert fp32 PSUM -> bf16 SBUF on their write path
    (free), halving the store stream to 16MB/core; the host upconverts
    to fp32 during the unshuffle.  Output rounding adds ~1e-3 to the
    scale-relative error (4.1e-3 total vs the 2e-2 gate).
    """
    nc = tc.nc
    with ExitStack() as ctx:
        pspool = ctx.enter_context(tc.tile_pool(name="psp", bufs=3,
                                                space="PSUM"))
        opool = ctx.enter_context(tc.tile_pool(name="op", bufs=3))

        for q in range(2):
            for r in range(4):
                ot = opool.tile([128, 4, 2, 1024], BF16, tag="ot")
                for b in range(2):
                    idx = q * 8 + r * 2 + b
                    for c in range(4):
                        psA = pspool.tile([128, 512], F32, tag="psA")
                        psB = pspool.tile([128, 512], F32, tag="psB")
                        dst = {0: psA[:, 0:256], 1: psA[:, 256:512],
                               2: psB[:, 0:256], 3: psB[:, 256:512]}
                        for dp in range(4):
                            d = (c + dp) % 4
                            a = 1 if c + dp < 4 else 0
                            nc.tensor.matmul(
                                dst[d],
                                lhsT=vvt[:, r * 2 + b, dp, :],
                                rhs=xall[:, idx, c, a:a + 256],
                                start=True, stop=True,
                            )
                        nc.scalar.activation(ot[:, c, b, 0:512], psA[:],
                                             AF.Copy)
                        nc.vector.tensor_copy(ot[:, c, b, 512:1024], psB[:])
                nc.sync.dma_start(out[q, r], ot[:])


def _emit(tc, xs, vv, out):
    with ExitStack() as ctx:
        vvt, xall = _emit_loads(ctx, tc, xs, vv)
        _emit_body(tc, vvt, xall, out)


_CACHE = {}


def _build_module(key):
    if key in _CACHE:
        return _CACHE[key]
    nc = bacc.Bacc("TRN2", target_bir_lowering=False, debug=False)
    xs = nc.dram_tensor("xs", [NB, KD, 4, PB], BF16, kind="ExternalInput").ap()
    vv = nc.dram_tensor("vv", [8, 4, KDP, 128], BF16,
                        kind="ExternalInput").ap()
    out = nc.dram_tensor("out", [2, 4, 128, 4, 2, 1024], BF16,
                         kind="ExternalOutput").ap()
    with tile.TileContext(nc) as tc:
        _emit(tc, xs, vv, out)
    nc.compile()
    _CACHE[key] = nc
    return nc


def _vmats(kv):
    """[8, 68, 128] f32 vertical interp matrices, index r*2+b (as before)."""
    V = np.zeros((8, KD, 128), np.float64)
    for r in range(4):
        for b in range(2):
            for m in range(128):
                d = (m - r) % 4
                p_lo = 32 * b + (m - r - d) // 4 + 1
                V[r * 2 + b, p_lo, m] += kv[3 - d]
                if d > 0:
                    V[r * 2 + b, p_lo + 1, m] += kv[7 - d]
    return V


def _vv_mats(kv, kh):
    """[8, 4, 68, 128] stacked matrices: rows 0-33 = w1*V34, 34-67 = w2*V34.

    V34 = V[r*2+b][32b : 32b+34]; (w1, w2) for horizontal phase offset d':
    (1,0), (.75,.25), (.5,.5), (.25,.75).
    """
    V = _vmats(kv)
    VV = np.zeros((8, 4, KDP, 128), np.float64)
    for r in range(4):
        for b in range(2):
            v34 = V[r * 2 + b, 32 * b: 32 * b + 34]
            for dp in range(4):
                w1 = float(kh[3 - dp])            # 1, .75, .5, .25
                w2 = float(kh[3 + 4 - dp]) if dp > 0 else 0.0
                VV[r * 2 + b, dp, 0:34] = w1 * v34
                VV[r * 2 + b, dp, 34:68] = w2 * v34
    return VV


def _slabs(x):
    s = np.zeros((N_CORES, SLAB, W), np.float32)
    for core in range(N_CORES):
        n, half = divmod(core, 2)
        g0 = 512 * half - 4
        s0, s1 = max(0, g0), min(H, g0 + SLAB)
        s[core, s0 - g0: s1 - g0] = x[n, 0, s0:s1]
    return s


def _xtiles(slab):
    """slab [528, 1024] f32 -> [16, 68, 4, 258] bf16 stacked tiles
    (rows 0-33 = contributing subrows; 34-67 = same, one subcol left)."""
    xt = np.zeros((2, 4, 2, KD, 4, PB), np.float32)
    for q in range(2):
        for r in range(4):
            for b in range(2):
                i0 = 64 * q + 32 * b
                rows = slab[4 * i0 + r: 4 * i0 + r + 4 * 34: 4]  # [34, 1024]
                bs = rows.reshape(34, 256, 4).transpose(0, 2, 1)  # [34,4,256]
                xt[q, r, b, 0:34, :, 1:257] = bs
                xt[q, r, b, 34:68, :, 0:256] = bs
    return xt.reshape(NB, KD, 4, PB).astype(BF)


_PERM = [rr * 4 + cc for (rr, cc) in OFFSETS]   # k -> flat (r, c) index


def _unshuffle(res):
    """Device out [2,4,128,4,2,4,256] (q,r,p,c,b,d,v) -> [16, 512, 1024]."""
    a = res.reshape(2, 4, 128, 4, 2, 4, 256)
    # target [k(r,c), row = 256q+128b+p, col = 4v+d]
    a = a.transpose(1, 3, 0, 4, 2, 6, 5)      # [r, c, q, b, p, v, d]
    a = np.ascontiguousarray(a).reshape(16, 512, 1024)
    return a[_PERM]


def kernel(x, weight):
    x = np.asarray(x, np.float32)
    weight = np.asarray(weight, np.float32)
    assert x.shape == (4, 1, H, W), x.shape
    k2 = weight[0, 0]
    kv = k2[:, 3].astype(np.float64)   # vertical profile
    kh = k2[3, :].astype(np.float64)   # horizontal profile

    nc = _build_module(tuple(np.asarray(k2, np.float64).ravel().tolist()))
    VV = _vv_mats(kv, kh).astype(BF)
    slabs = _slabs(x)
    in_maps = [{"xs": _xtiles(slabs[c]), "vv": VV} for c in range(N_CORES)]
    res = run_bass_kernel_spmd(nc, in_maps, list(range(N_CORES)))

    full = np.empty((4, 16, H, W), np.float32)
    for core in range(N_CORES):
        n, half = divmod(core, 2)
        full[n, :, 512 * half: 512 * half + 512, :] = \
            _unshuffle(np.asarray(res.results[core]["out"], np.float32))
    return full
